# revision 55
# baseline (speedup 1.0000x reference)
"""Causal self-attention (B=4, S=4096, D=64, H=4) on 8 TRN2 NeuronCores.

Sharding: 16 (batch, head) pairs, 2 per core (core c -> batch c//2,
heads (2*(c%2), 2*(c%2)+1)). Each core runs fused attention for its 2
pairs; no cross-core communication.

Per-core program (SPMD). Key structure vs a naive port:
  - Q/K projections are folded into the score matmul: host precomputes
    M_h = Wk_h^T Wq_h / sqrt(Dh) per head, the kernel computes
    Z = M x once (K=64) and scores S^T = x^T Z directly (K=64
    contraction, 2-band row tiling at partition offsets 0/64).
    Query-side bias terms are softmax-invariant and dropped; the
    key-side bias term exp(C_k), C_k = bq.(Wk x_k)/sqrt(Dh), is folded
    into a host-prescaled copy of x used for the V projection (x2v),
    and bv is added on the host after the final division.
  - scores are computed TRANSPOSED (key position on partitions) so P@V
    needs no transpose; the softmax denominator comes from an extra
    "ones" column in the V projection (which also carries exp(C_k)).
  - exp of the score matrix is split between the ACT engine (exact Exp)
    and the DVE (Schraudolph-style bit-trick: y_i16 = trunc(A*s + B)
    bitcast to bf16 ~ exp(s), max rel err ~3.3%), load-balanced, since
    ACT alone is a ~113us floor at 1.2 GHz x 128 lanes.
  - causal mask: -30 additive on the 4 diagonal 128x128 tiles per
    query superblock, applied by the PE itself as an accumulating
    matmul (stationary = I, moving = -30*strict_lower_tri), freeing
    ACT/DVE. Below-diagonal garbage is excluded from PV by qoff.
  - PV matmuls are 2-way column-tiled (out M=17 <= 32, col groups at
    partition 0/32 of one PSUM bank); the two partial sums are reduced
    and copied to SBUF in one DVE scalar_tensor_tensor, then DMA'd out.
  - The PE stream is kept dense (software-pipelined PV of the previous
    unit between score chunks) so the HAM clock gate stays at 2.4 GHz.
  - output: [2 pairs, 17, 4096] f32 = unnormalized O^T rows 0..15 plus
    the softmax denominator in row 16; division + bv happen on host.
"""

import numpy as np
import ml_dtypes

_B, _S, _D = 4, 4096, 64
_H, _Dh = 4, 16
_NC = 8
_NQB = _S // 512  # 8 query superblocks of 512
_NKB = _S // 128  # 32 key blocks of 128
_CHUNK = 3  # off-diag key-blocks per psum chunk (3 banks)
_MASK = -30.0
_EXP_A = 184.6649652337873  # 128/ln2
_EXP_B = 16250.9  # 128*127 - c, calibrated for trunc, min-max rel err

# diag chunk psum packing: block j at col offs[j], stream length lens[j],
# starting at query qstart[j] (within the 512-superblock); the causal
# triangle of block j sits at col moff[j].
_D_OFF = (0, 512, 1024, 1280)
_D_LEN = (512, 512, 256, 128)
_D_QS = (0, 0, 256, 384)
_D_MOFF = (0, 640, 1024, 1280)
# PV moving slice for diag block j: pt col offset (rel. to diag region
# start), length, and po4 query offset
_D_PV = ((0, 512, 0), (640, 384, 128), (1024, 256, 256), (1280, 128, 384))

_cache = {}


def _build_nc():
    import concourse.tile as tile
    from concourse import bacc, mybir

    bf = mybir.dt.bfloat16
    i16 = mybir.dt.int16
    f32 = mybir.dt.float32
    Exp = mybir.ActivationFunctionType.Exp
    MUL = mybir.AluOpType.mult
    ADD = mybir.AluOpType.add

    nc = bacc.Bacc("TRN2", target_bir_lowering=False, debug=False, num_devices=_NC)
    x2_d = nc.dram_tensor("x2", [128, _S], bf, kind="ExternalInput").ap()
    zw_d = nc.dram_tensor("zw", [_D, 128], bf, kind="ExternalInput").ap()
    x2v_d = nc.dram_tensor("x2v", [2, _D + 1, _S], bf, kind="ExternalInput").ap()
    wv_d = nc.dram_tensor("wv", [_D + 1, 34], bf, kind="ExternalInput").ap()
    tmv_d = nc.dram_tensor("tmv", [128, 128], bf, kind="ExternalInput").ap()
    i128_d = nc.dram_tensor("i128", [128, 128], bf, kind="ExternalInput").ap()
    # per (pair, superblock): rows 0-16 = PV partial sum of even key
    # blocks (16 v-dims + denominator), rows 32-48 = odd partial; host
    # adds the two partials and divides.
    out_d = nc.dram_tensor("out", [2, _NQB, 128, 512], f32, kind="ExternalOutput").ap()

    # static greedy load balance between ACT ("scalar") and DVE ("vector"),
    # with hysteresis: consecutive chunks prefer to alternate engines so the
    # 2-deep PSUM chunk pipeline never stalls behind one busy engine
    load = {"a": 0.0, "v": 0.0}
    last_eng = [""]

    def exp_cost(eng, cols):
        return (cols + 430) * 0.833 if eng == "a" else (cols + 300) * 1.042

    def pick_engine(cols):
        ca, cv = exp_cost("a", cols), exp_cost("v", cols)
        sa = load["a"] + ca + (1500.0 if last_eng[0] == "a" else 0.0)
        sv = load["v"] + cv + (1500.0 if last_eng[0] == "v" else 0.0)
        e = "a" if sa <= sv else "v"
        load[e] += exp_cost(e, cols)
        last_eng[0] = e
        return e

    with tile.TileContext(nc) as tc:
        with tc.tile_pool(name="singles", bufs=1) as singles:
            x2 = singles.tile([128, _S], bf, tag="x2")
            zw = singles.tile([_D, 128], bf, tag="zw")
            x2v = [
                singles.tile([_D + 1, _S], bf, tag=f"x2v{p}", name=f"x2v{p}")
                for p in range(2)
            ]
            wv = singles.tile([_D + 1, 34], bf, tag="wv")
            tmv = singles.tile([128, 128], bf, tag="tmv")
            i128 = singles.tile([128, 128], bf, tag="i128")
            z2 = [
                singles.tile([128, _S], bf, tag=f"z2{p}", name=f"z2{p}")
                for p in range(2)
            ]
            # V layout: slot (b, p) at index 2b+p, 32 cols each: 17 real
            # (16 v-dims + denominator) + 15 zeros. The even PV col-group
            # uses the full 32-wide slot so its output also zero-fills po
            # rows 17..31 (which the drain copy reads).
            vall = singles.tile([128, 2 * _NKB, 32], bf, tag="vall")
            nc.vector.memset(vall[:], 0.0)

            # spread input DMAs over 4 DGE queues; per queue: small consts
            # first, then an x2 quarter (needed by the first scores), then
            # an x2v half (needed by the deferred V projection)
            # 3 usable DGE queues. x2 goes in 8 pieces, HIGH columns first
            # (the first unit consumes keys high-to-low and the Z projection
            # needs the last 512 columns), so the PE can start ~3us in.
            # x2v (only needed by the deferred V projection) follows.
            qs = [nc.sync, nc.gpsimd, nc.scalar]
            nc.sync.dma_start(out=zw[:], in_=zw_d)
            nc.gpsimd.dma_start(out=wv[:], in_=wv_d)
            nc.scalar.dma_start(out=tmv[:], in_=tmv_d)
            nc.scalar.dma_start(out=i128[:], in_=i128_d)
            for i, s in enumerate(reversed(range(8))):
                qs[i % 3].dma_start(
                    out=x2[:, 512 * s : 512 * (s + 1)],
                    in_=x2_d[:, 512 * s : 512 * (s + 1)],
                )
            for p in range(2):
                for c in range(2):
                    qs[(2 * p + c) % 3].dma_start(
                        out=x2v[p][:, 2048 * c : 2048 * (c + 1)],
                        in_=x2v_d[p][:, 2048 * c : 2048 * (c + 1)],
                    )

            def copy_psum(dst, src, cols):
                """psum->sbuf copy on the less-loaded of ACT/DVE."""
                e = pick_engine(cols)
                if e == "a":
                    nc.scalar.copy(dst, src)
                else:
                    nc.vector.tensor_copy(dst, src)

            # one PSUM pool: tag "sc" 2x3 banks + tag "po" 2x1 banks = 8
            # banks; Z/V projection tiles ride the "po" ring (same shape,
            # disjoint lifetime phases).
            with (
                tc.tile_pool(name="psp", bufs=2, space="PSUM") as psp,
                tc.tile_pool(name="ptp", bufs=2) as ptp,
                tc.tile_pool(name="stg", bufs=3) as stg,
            ):
                def emit_z(p, qi):
                    """Z2_p chunk qi: Z = M x, written twice (col-tiled) so
                    the copy lands band-replicated at partitions 0/64."""
                    csl = slice(512 * qi, 512 * (qi + 1))
                    zt = psp.tile([128, 512], f32, tag="ztv", name="zt", bufs=1)
                    nc.tensor.matmul(
                        zt[0:64, :],
                        zw[:, 64 * p : 64 * p + 64],
                        x2[0:64, csl],
                        start=True,
                        stop=True,
                        tile_position=(0, 0),
                    )
                    nc.tensor.matmul(
                        zt[64:128, :],
                        zw[:, 64 * p : 64 * p + 64],
                        x2[0:64, csl],
                        start=True,
                        stop=True,
                        tile_position=(0, 64),
                    )
                    copy_psum(z2[p][:, csl], zt[:], 512)

                # Pre-warm the PE: the HAM clock gate needs ~3.4us of
                # sustained activity to release the 1.2 GHz throttle. These
                # dummy matmuls run while the input DMAs land, so real work
                # starts at 2.4 GHz.
                warm = singles.tile([128, 16], bf, tag="warm")
                nc.vector.memset(warm[:], 0.0)
                wps = psp.tile([128, 512], f32, tag="po", name="wps", bufs=1)
                for _ in range(40):
                    nc.tensor.matmul(
                        wps[0:16, 0:16], warm[:], warm[:], start=True, stop=True
                    )

                # Z chunks for the first units; the remaining Z chunks are
                # emitted one per unit inside the loop.
                emit_z(0, _NQB - 1)
                emit_z(1, _NQB - 1)

                # V projection: out[k, col] = sum_d x2v_p[d,k] * wv[d,col];
                # col 16 of each pair block is the (exp(C_k)) denominator.
                # Deferred into the first unit's emission so the x2v input
                # DMA overlaps with the first score chunks.
                _vt_nb = [15, 15, 2]

                def emit_vproj(t):
                    nb = _vt_nb[t]
                    b0v = sum(_vt_nb[:t])
                    vt = psp.tile([128, 512], f32, tag="ztv", name="vt", bufs=1)
                    for i in range(nb):
                        b = b0v + i
                        for p in range(2):
                            nc.tensor.matmul(
                                vt[:, 34 * i + 17 * p : 34 * i + 17 * p + 17],
                                x2v[p][:, 128 * b : 128 * (b + 1)],
                                wv[:, 17 * p : 17 * p + 17],
                                start=True,
                                stop=True,
                                skip_group_check=True,
                            )
                    copy_psum(
                        vall[:, 2 * b0v : 2 * (b0v + nb), 0:17],
                        vt[:, : 34 * nb].rearrange("p (b c) -> p b c", c=17),
                        34 * nb,
                    )

                def emit_exp(ps_ap, pt_ap, cols):
                    e = pick_engine(cols)
                    if e == "a":
                        nc.scalar.activation(out=pt_ap, in_=ps_ap, func=Exp)
                    else:
                        nc.vector.tensor_scalar(
                            pt_ap.bitcast(i16),
                            ps_ap,
                            _EXP_A,
                            _EXP_B,
                            MUL,
                            ADD,
                        )

                out_q = [nc.sync, nc.gpsimd]

                class BUnit:
                    """PV matmuls (4-way col-tiled partial sums; the host
                    adds the partials) + drain copy + output DMA, emitted
                    incrementally so they interleave with the next unit's
                    score chunks."""

                    n_out = [0]

                    def __init__(self, p, qi, pt, tag="po"):
                        self.p, self.qi, self.pt = p, qi, pt
                        self.nkb = 4 * qi + 4
                        self.done = 0
                        self.po = psp.tile([128, 512], f32, tag=tag, name="po", bufs=1)

                    def emit_upto(self, k):
                        # col-group count: 4 for qi>=1, 2 for the all-diag
                        # qi=0 units (their odd groups would leave column
                        # gaps with 4-way tiling)
                        ng = 4 if self.qi >= 1 else 2
                        for b in range(self.done, min(k, self.nkb)):
                            j = b - 4 * self.qi
                            if j < 0:
                                off, ln, qo = 512 * b - 2048 * self.qi, 512, 0
                            else:
                                off, ln, qo = _D_PV[j]
                                if self.qi == 0 and j == 1:
                                    # cover queries 0..127 too (garbage there;
                                    # host ignores) so the drain copy's read
                                    # region is fully written this generation
                                    off, ln, qo = 512, 512, 0
                            base = 2048 * self.qi + off
                            msl = self.pt[:, base : base + ln]
                            vsl = vall[:, 2 * b + self.p]
                            # each col-group uses the 32-wide zero-padded
                            # stationary so its full 32-row band is written;
                            # group 0 carries the sim's group bookkeeping
                            g = b % ng
                            nc.tensor.matmul(
                                self.po[32 * g : 32 * g + 32, qo:512],
                                vsl,
                                msl,
                                start=(b < ng),
                                stop=(b == self.nkb - ng),
                                skip_group_check=(g != 0),
                                tile_position=(0, 32 * g),
                            )
                        self.done = max(self.done, min(k, self.nkb))

                    def finish(self):
                        self.emit_upto(self.nkb)
                        nrow = 128 if self.qi >= 1 else 64
                        ost = stg.tile([128, 512], f32, tag="ost", name="ost")
                        copy_psum(ost[0:nrow, :], self.po[0:nrow, :], 512)
                        q = out_q[BUnit.n_out[0] % len(out_q)]
                        BUnit.n_out[0] += 1
                        q.dma_start(
                            out=out_d[self.p][self.qi][0:nrow], in_=ost[0:nrow, :]
                        )

                def emit_offdiag_chunk(p, qi, pt, b0, nblk):
                    ps = psp.tile([128, 512 * _CHUNK], f32, tag="sc", name="ps")
                    for t in range(nblk):
                        b = b0 + t
                        g = 64 * (b % 2)
                        nc.tensor.matmul(
                            ps[:, 512 * t : 512 * (t + 1)],
                            x2[g : g + 64, 128 * b : 128 * (b + 1)],
                            z2[p][g : g + 64, 512 * qi : 512 * (qi + 1)],
                            start=True,
                            stop=True,
                            tile_position=(g, 0),
                        )
                    emit_exp(
                        ps[:, : 512 * nblk],
                        pt[:, 512 * b0 : 512 * (b0 + nblk)],
                        512 * nblk,
                    )

                def emit_diag_chunk(p, qi, pt):
                    bd = 4 * qi
                    ps = psp.tile([128, 512 * _CHUNK], f32, tag="sc", name="ps")

                    # scores j0..j2, then their masks, then j3 + its mask:
                    # j3's start re-marks bank 2 pending, so mask-j2 must
                    # accumulate before j3 runs.
                    def diag_score(j):
                        b = bd + j
                        g = 64 * (b % 2)
                        qs = 512 * qi + _D_QS[j]
                        nc.tensor.matmul(
                            ps[:, _D_OFF[j] : _D_OFF[j] + _D_LEN[j]],
                            x2[g : g + 64, 128 * b : 128 * (b + 1)],
                            z2[p][g : g + 64, qs : qs + _D_LEN[j]],
                            start=True,
                            stop=True,
                            tile_position=(g, 0),
                            skip_group_check=(j == 3),
                        )

                    def diag_mask(j):
                        nc.tensor.matmul(
                            ps[:, _D_MOFF[j] : _D_MOFF[j] + 128],
                            i128[:],
                            tmv[:, 0:128],
                            start=False,
                            stop=False,
                            skip_group_check=True,
                        )

                    for j in range(3):
                        diag_score(j)
                    for j in range(3):
                        diag_mask(j)
                    diag_score(3)
                    diag_mask(3)
                    emit_exp(ps[:, :1408], pt[:, 512 * bd : 512 * bd + 1408], 1408)

                units = [(p, qi) for qi in reversed(range(_NQB)) for p in range(2)]
                prev = None
                for ui, (p, qi) in enumerate(units):
                    bd = 4 * qi
                    # chunk list: (kind, b0, nblk, covered-upto)
                    offs = [
                        ("off", b0, min(_CHUNK, bd - b0))
                        for b0 in range(0, bd, _CHUNK)
                    ]
                    if ui == 0:
                        # consume keys high-to-low: matches the descending
                        # input DMA order, so compute starts immediately
                        chunks = [("diag", bd, 4)] + offs[::-1]
                    else:
                        chunks = offs + [("diag", bd, 4)]
                    nchunks = len(chunks)
                    pt = ptp.tile([128, 512 * _NKB], bf, tag="pt", name="pt")
                    if qi > 0:
                        emit_z(p, qi - 1)  # consumed two units later
                    last = ui == len(units) - 1
                    if last:
                        # self-paced drain on the spare ztv psum slot: PVs
                        # chase this unit's own exp chunk by chunk
                        me = BUnit(p, qi, pt, tag="ztv")
                        mydone = 0
                    for ci, (kind, b0, nblk) in enumerate(chunks):
                        if prev is not None:
                            prev.emit_upto((ci * prev.nkb) // nchunks)
                        if kind == "off":
                            emit_offdiag_chunk(p, qi, pt, b0, nblk)
                        else:
                            emit_diag_chunk(p, qi, pt)
                        if ui == 0 and ci == 8:
                            emit_vproj(0)
                        if ui == 1 and ci in (2, 5):
                            emit_vproj(1 if ci == 2 else 2)
                        if last:
                            mydone += nblk
                            me.emit_upto(mydone)
                    if prev is not None:
                        prev.finish()
                    if last:
                        me.finish()
                    else:
                        prev = BUnit(p, qi, pt)

    nc.compile()
    return nc


def _get_nc():
    if "nc" not in _cache:
        _cache["nc"] = _build_nc()
    return _cache["nc"]


def _prepare_in_maps(x, Wq, bq, Wk, bk, Wv, bv):
    bfd = ml_dtypes.bfloat16
    x = np.asarray(x, np.float32)
    Wq = np.asarray(Wq, np.float32)
    bq = np.asarray(bq, np.float32)
    Wk = np.asarray(Wk, np.float32)
    Wv = np.asarray(Wv, np.float32)

    tmv = np.where(
        np.arange(128)[:, None] > np.arange(128)[None, :], _MASK, 0.0
    ).astype(np.float32)
    i128 = np.eye(128, dtype=np.float32)

    in_maps = []
    for c in range(_NC):
        b_idx = c // 2
        heads = (2 * (c % 2), 2 * (c % 2) + 1)
        xT = x[b_idx].T  # [64, 4096]
        x2 = np.concatenate([xT, xT], axis=0)  # [128, 4096]
        zw = np.zeros((_D, 128), np.float32)
        x2v = np.zeros((2, _D + 1, _S), np.float32)
        wv = np.zeros((_D + 1, 34), np.float32)
        for p, h in enumerate(heads):
            hs = slice(h * _Dh, (h + 1) * _Dh)
            Wqh, Wkh, Wvh = Wq[hs], Wk[hs], Wv[hs]
            M = Wkh.T @ Wqh / np.sqrt(_Dh)  # [64, 64]
            zw[:, 64 * p : 64 * p + 64] = M.T
            C = (bq[hs] @ (Wkh @ xT)) / np.sqrt(_Dh)  # [4096]
            expC = np.exp(C).astype(np.float32)
            x2v[p, :_D] = xT * expC[None, :]
            x2v[p, _D] = expC
            wv[:_D, 17 * p : 17 * p + 16] = Wvh.T
            wv[_D, 17 * p + 16] = 1.0
        in_maps.append(
            {
                "x2": x2.astype(bfd),
                "zw": zw.astype(bfd),
                "x2v": x2v.astype(bfd),
                "wv": wv.astype(bfd),
                "tmv": tmv.astype(bfd),
                "i128": i128.astype(bfd),
            }
        )
    return in_maps


def _assemble(results, bv):
    bv = np.asarray(bv, np.float32)
    final = np.empty((_B, _S, _D), np.float32)
    for c in range(_NC):
        b_idx = c // 2
        o = np.asarray(results[c]["out"], np.float32)  # [2, NQB, 128, 512]
        for p in range(2):
            h = 2 * (c % 2) + p
            hs = slice(h * _Dh, (h + 1) * _Dh)
            # qi>=1: four col-group partials; qi=0: two, and the second
            # never covers queries 0..127 (use the first alone there)
            part = o[p, :, 0:17] + o[p, :, 32:49]  # [NQB, 17, 512]
            part[1:] += o[p, 1:, 64:81] + o[p, 1:, 96:113]
            part[0, :, 0:128] = o[p, 0, 0:17, 0:128]
            ot = part.transpose(1, 0, 2).reshape(17, _S)  # [17, S]
            final[b_idx, :, hs] = (ot[:16] / ot[16:17]).T + bv[hs][None, :]
    return final


def _run(in_maps, trace=False, trace_kwargs=None):
    from concourse.bass_utils import run_bass_kernel_spmd

    nc = _get_nc()
    return run_bass_kernel_spmd(
        nc, in_maps, list(range(_NC)), trace=trace, **(trace_kwargs or {})
    )


def kernel(x, Wq, bq, Wk, bk, Wv, bv):
    in_maps = _prepare_in_maps(x, Wq, bq, Wk, bk, Wv, bv)
    res = _run(in_maps)
    return _assemble(res.results, bv)


# revision 62
# speedup vs baseline: 1.2021x; 1.2021x over previous
"""Causal self-attention (B=4, S=4096, D=64, H=4) on 8 TRN2 NeuronCores.

Sharding: 16 (batch, head) pairs, 2 per core (core c -> batch c//2,
heads (2*(c%2), 2*(c%2)+1)). Each core runs fused attention for its 2
pairs; no cross-core communication.

Per-core program (SPMD). Key structure vs a naive port:
  - Q/K projections are folded into the score matmul: host precomputes
    M_h = Wk_h^T Wq_h / sqrt(Dh) per head, the kernel computes
    Z = M x once (K=64) and scores S^T = x^T Z directly (K=64
    contraction, 2-band row tiling at partition offsets 0/64).
    Query-side bias terms are softmax-invariant and dropped; the
    key-side bias term exp(C_k), C_k = bq.(Wk x_k)/sqrt(Dh), is folded
    into a host-prescaled copy of x used for the V projection (x2v),
    and bv is added on the host after the final division.
  - scores are computed TRANSPOSED (key position on partitions) so P@V
    needs no transpose; the softmax denominator comes from an extra
    "ones" column in the V projection (which also carries exp(C_k)).
  - exp of the score matrix is split between the ACT engine (exact Exp)
    and the DVE (Schraudolph-style bit-trick: y_i16 = trunc(A*s + B)
    bitcast to bf16 ~ exp(s), max rel err ~3.3%), load-balanced, since
    ACT alone is a ~113us floor at 1.2 GHz x 128 lanes.
  - causal mask: -30 additive on the 4 diagonal 128x128 tiles per
    query superblock, applied by the PE itself as an accumulating
    matmul (stationary = I, moving = -30*strict_lower_tri), freeing
    ACT/DVE. Below-diagonal garbage is excluded from PV by qoff.
  - PV matmuls are 2-way column-tiled (out M=17 <= 32, col groups at
    partition 0/32 of one PSUM bank); the two partial sums are reduced
    and copied to SBUF in one DVE scalar_tensor_tensor, then DMA'd out.
  - The PE stream is kept dense (software-pipelined PV of the previous
    unit between score chunks) so the HAM clock gate stays at 2.4 GHz.
  - output: [2 pairs, 17, 4096] f32 = unnormalized O^T rows 0..15 plus
    the softmax denominator in row 16; division + bv happen on host.
"""

import numpy as np
import ml_dtypes

_B, _S, _D = 4, 4096, 64
_H, _Dh = 4, 16
_NC = 8
_NQB = _S // 512  # 8 query superblocks of 512
_NKB = _S // 128  # 32 key blocks of 128
_CHUNK = 2  # off-diag key-blocks per psum chunk (2 banks, 3-deep pipeline)
_MASK = -30.0
_EXP_A = 184.6649652337873  # 128/ln2
_EXP_B = 16250.9  # 128*127 - c, calibrated for trunc, min-max rel err

# diag chunk psum packing: block j at col offs[j], stream length lens[j],
# starting at query qstart[j] (within the 512-superblock); the causal
# triangle of block j sits at col moff[j].
_D_OFF = (0, 512, 1024, 1280)
_D_LEN = (512, 512, 256, 128)
_D_QS = (0, 0, 256, 384)
_D_MOFF = (0, 640, 1024, 1280)
# PV moving slice for diag block j: pt col offset (rel. to diag region
# start), length, and po4 query offset
_D_PV = ((0, 512, 0), (640, 384, 128), (1024, 256, 256), (1280, 128, 384))

_cache = {}


def _build_nc():
    import concourse.tile as tile
    from concourse import bacc, mybir

    bf = mybir.dt.bfloat16
    i16 = mybir.dt.int16
    f32 = mybir.dt.float32
    Exp = mybir.ActivationFunctionType.Exp
    MUL = mybir.AluOpType.mult
    ADD = mybir.AluOpType.add

    nc = bacc.Bacc("TRN2", target_bir_lowering=False, debug=False, num_devices=_NC)
    x2_d = nc.dram_tensor("x2", [128, _S], bf, kind="ExternalInput").ap()
    zw_d = nc.dram_tensor("zw", [_D, 128], bf, kind="ExternalInput").ap()
    x2v_d = nc.dram_tensor("x2v", [2, _D + 1, _S], bf, kind="ExternalInput").ap()
    wv_d = nc.dram_tensor("wv", [_D + 1, 34], bf, kind="ExternalInput").ap()
    tmv_d = nc.dram_tensor("tmv", [128, 128], bf, kind="ExternalInput").ap()
    i128_d = nc.dram_tensor("i128", [128, 128], bf, kind="ExternalInput").ap()
    # per (pair, superblock): rows 0-16 = PV partial sum of even key
    # blocks (16 v-dims + denominator), rows 32-48 = odd partial; host
    # adds the two partials and divides.
    out_d = nc.dram_tensor("out", [2, _NQB, 128, 512], f32, kind="ExternalOutput").ap()

    # static greedy load balance between ACT ("scalar") and DVE ("vector")
    load = {"a": 0.0, "v": 0.0}

    def exp_cost(eng, cols):
        return (cols + 430) * 0.833 if eng == "a" else (cols + 300) * 1.042

    def pick_engine(cols):
        ca, cv = exp_cost("a", cols), exp_cost("v", cols)
        e = "a" if load["a"] + ca <= load["v"] + cv else "v"
        load[e] += exp_cost(e, cols)
        return e

    with tile.TileContext(nc) as tc:
        with tc.tile_pool(name="singles", bufs=1) as singles:
            x2 = singles.tile([128, _S], bf, tag="x2")
            zw = singles.tile([_D, 128], bf, tag="zw")
            x2v = [
                singles.tile([_D + 1, _S], bf, tag=f"x2v{p}", name=f"x2v{p}")
                for p in range(2)
            ]
            wv = singles.tile([_D + 1, 34], bf, tag="wv")
            tmv = singles.tile([128, 128], bf, tag="tmv")
            i128 = singles.tile([128, 128], bf, tag="i128")
            z2 = [
                singles.tile([128, _S], bf, tag=f"z2{p}", name=f"z2{p}")
                for p in range(2)
            ]
            # V layout: slot (b, p) at index 2b+p, 32 cols each: 17 real
            # (16 v-dims + denominator) + 15 zeros. The even PV col-group
            # uses the full 32-wide slot so its output also zero-fills po
            # rows 17..31 (which the drain copy reads).
            vall = singles.tile([128, 2 * _NKB, 32], bf, tag="vall")
            nc.vector.memset(vall[:], 0.0)

            # spread input DMAs over 4 DGE queues; per queue: small consts
            # first, then an x2 quarter (needed by the first scores), then
            # an x2v half (needed by the deferred V projection)
            # 3 usable DGE queues. x2 goes in 8 pieces, HIGH columns first
            # (the first unit consumes keys high-to-low and the Z projection
            # needs the last 512 columns), so the PE can start ~3us in.
            # x2v (only needed by the deferred V projection) follows.
            qs = [nc.sync, nc.gpsimd, nc.scalar]
            nc.sync.dma_start(out=zw[:], in_=zw_d)
            nc.gpsimd.dma_start(out=wv[:], in_=wv_d)
            nc.scalar.dma_start(out=tmv[:], in_=tmv_d)
            nc.scalar.dma_start(out=i128[:], in_=i128_d)
            for i, s in enumerate(reversed(range(8))):
                qs[i % 3].dma_start(
                    out=x2[:, 512 * s : 512 * (s + 1)],
                    in_=x2_d[:, 512 * s : 512 * (s + 1)],
                )
            for p in range(2):
                for c in range(2):
                    qs[(2 * p + c) % 3].dma_start(
                        out=x2v[p][:, 2048 * c : 2048 * (c + 1)],
                        in_=x2v_d[p][:, 2048 * c : 2048 * (c + 1)],
                    )

            def copy_psum(dst, src, cols):
                """psum->sbuf copy on the less-loaded of ACT/DVE."""
                e = pick_engine(cols)
                if e == "a":
                    nc.scalar.copy(dst, src)
                else:
                    nc.vector.tensor_copy(dst, src)

            # one PSUM pool: tag "sc" 3x2 banks + "po" 1 + "ztv" 1 = 8 banks
            with (
                tc.tile_pool(name="psp", bufs=3, space="PSUM") as psp,
                tc.tile_pool(name="ptp", bufs=2) as ptp,
                tc.tile_pool(name="stg", bufs=3) as stg,
            ):
                def emit_z(p, qi):
                    """Z2_p chunk qi: Z = M x, written twice (col-tiled) so
                    the copy lands band-replicated at partitions 0/64."""
                    csl = slice(512 * qi, 512 * (qi + 1))
                    zt = psp.tile([128, 512], f32, tag="ztv", name="zt", bufs=1)
                    nc.tensor.matmul(
                        zt[0:64, :],
                        zw[:, 64 * p : 64 * p + 64],
                        x2[0:64, csl],
                        start=True,
                        stop=True,
                        tile_position=(0, 0),
                    )
                    nc.tensor.matmul(
                        zt[64:128, :],
                        zw[:, 64 * p : 64 * p + 64],
                        x2[0:64, csl],
                        start=True,
                        stop=True,
                        tile_position=(0, 64),
                    )
                    copy_psum(z2[p][:, csl], zt[:], 512)

                # Pre-warm the PE: the HAM clock gate needs ~3.4us of
                # sustained activity to release the 1.2 GHz throttle. These
                # dummy matmuls run while the input DMAs land, so real work
                # starts at 2.4 GHz.
                warm = singles.tile([128, 16], bf, tag="warm")
                nc.vector.memset(warm[:], 0.0)
                wps = psp.tile([128, 512], f32, tag="po", name="wps", bufs=1)
                for _ in range(40):
                    nc.tensor.matmul(
                        wps[0:16, 0:16], warm[:], warm[:], start=True, stop=True
                    )

                # Z chunks for the first units; the remaining Z chunks are
                # emitted one per unit inside the loop.
                emit_z(0, _NQB - 1)
                emit_z(1, _NQB - 1)

                # V projection: out[k, col] = sum_d x2v_p[d,k] * wv[d,col];
                # col 16 of each pair block is the (exp(C_k)) denominator.
                # Deferred into the first unit's emission so the x2v input
                # DMA overlaps with the first score chunks.
                _vt_nb = [15, 15, 2]

                def emit_vproj(t):
                    nb = _vt_nb[t]
                    b0v = sum(_vt_nb[:t])
                    vt = psp.tile([128, 512], f32, tag="ztv", name="vt", bufs=1)
                    for i in range(nb):
                        b = b0v + i
                        for p in range(2):
                            nc.tensor.matmul(
                                vt[:, 34 * i + 17 * p : 34 * i + 17 * p + 17],
                                x2v[p][:, 128 * b : 128 * (b + 1)],
                                wv[:, 17 * p : 17 * p + 17],
                                start=True,
                                stop=True,
                                skip_group_check=True,
                            )
                    copy_psum(
                        vall[:, 2 * b0v : 2 * (b0v + nb), 0:17],
                        vt[:, : 34 * nb].rearrange("p (b c) -> p b c", c=17),
                        34 * nb,
                    )

                def emit_exp(ps_ap, pt_ap, cols):
                    e = pick_engine(cols)
                    if e == "a":
                        nc.scalar.activation(out=pt_ap, in_=ps_ap, func=Exp)
                    else:
                        nc.vector.tensor_scalar(
                            pt_ap.bitcast(i16),
                            ps_ap,
                            _EXP_A,
                            _EXP_B,
                            MUL,
                            ADD,
                        )

                out_q = [nc.sync, nc.gpsimd]

                class BUnit:
                    """PV matmuls (4-way col-tiled partial sums; the host
                    adds the partials) + drain copy + output DMA, emitted
                    incrementally so they interleave with the next unit's
                    score chunks."""

                    n_out = [0]

                    def __init__(self, p, qi, pt, tag="po"):
                        self.p, self.qi, self.pt = p, qi, pt
                        self.nkb = 4 * qi + 4
                        self.done = 0
                        self.po = psp.tile([128, 512], f32, tag=tag, name="po", bufs=1)

                    def emit_upto(self, k):
                        # col-group count: 4 for qi>=1, 2 for the all-diag
                        # qi=0 units (their odd groups would leave column
                        # gaps with 4-way tiling)
                        ng = 4 if self.qi >= 1 else 2
                        for b in range(self.done, min(k, self.nkb)):
                            j = b - 4 * self.qi
                            if j < 0:
                                off, ln, qo = 512 * b - 2048 * self.qi, 512, 0
                            else:
                                off, ln, qo = _D_PV[j]
                                if self.qi == 0 and j == 1:
                                    # cover queries 0..127 too (garbage there;
                                    # host ignores) so the drain copy's read
                                    # region is fully written this generation
                                    off, ln, qo = 512, 512, 0
                            base = 2048 * self.qi + off
                            msl = self.pt[:, base : base + ln]
                            vsl = vall[:, 2 * b + self.p]
                            # each col-group uses the 32-wide zero-padded
                            # stationary so its full 32-row band is written;
                            # group 0 carries the sim's group bookkeeping
                            g = b % ng
                            nc.tensor.matmul(
                                self.po[32 * g : 32 * g + 32, qo:512],
                                vsl,
                                msl,
                                start=(b < ng),
                                stop=(b == self.nkb - ng),
                                skip_group_check=(g != 0),
                                tile_position=(0, 32 * g),
                            )
                        self.done = max(self.done, min(k, self.nkb))

                    def finish(self):
                        self.emit_upto(self.nkb)
                        nrow = 128 if self.qi >= 1 else 64
                        ost = stg.tile([128, 512], f32, tag="ost", name="ost")
                        copy_psum(ost[0:nrow, :], self.po[0:nrow, :], 512)
                        q = out_q[BUnit.n_out[0] % len(out_q)]
                        BUnit.n_out[0] += 1
                        q.dma_start(
                            out=out_d[self.p][self.qi][0:nrow], in_=ost[0:nrow, :]
                        )

                def emit_offdiag_chunk(p, qi, pt, b0, nblk):
                    ps = psp.tile([128, 512 * _CHUNK], f32, tag="sc", name="ps")
                    for t in range(nblk):
                        b = b0 + t
                        g = 64 * (b % 2)
                        nc.tensor.matmul(
                            ps[:, 512 * t : 512 * (t + 1)],
                            x2[g : g + 64, 128 * b : 128 * (b + 1)],
                            z2[p][g : g + 64, 512 * qi : 512 * (qi + 1)],
                            start=True,
                            stop=True,
                            tile_position=(g, 0),
                        )
                    emit_exp(
                        ps[:, : 512 * nblk],
                        pt[:, 512 * b0 : 512 * (b0 + nblk)],
                        512 * nblk,
                    )

                def emit_diag_chunk(p, qi, pt, half):
                    """Diagonal scores in two 2-bank chunks: half 0 = blocks
                    j0 (full 512) + j1 (full 512, garbage below the
                    staircase, PV skips it); half 1 = j2 (256) + j3 (128).
                    Masks: triangle of block j accumulates right after its
                    score; within a shared bank the mask must precede the
                    next score's start=True re-marking."""
                    bd = 4 * qi
                    ps = psp.tile([128, 512 * _CHUNK], f32, tag="sc", name="ps")

                    def diag_score(j, off, ln):
                        b = bd + j
                        g = 64 * (b % 2)
                        qs = 512 * qi + _D_QS[j]
                        nc.tensor.matmul(
                            ps[:, off : off + ln],
                            x2[g : g + 64, 128 * b : 128 * (b + 1)],
                            z2[p][g : g + 64, qs : qs + ln],
                            start=True,
                            stop=True,
                            tile_position=(g, 0),
                        )

                    def diag_mask(off):
                        nc.tensor.matmul(
                            ps[:, off : off + 128],
                            i128[:],
                            tmv[:, 0:128],
                            start=False,
                            stop=False,
                            skip_group_check=True,
                        )

                    if half == 0:
                        diag_score(0, 0, 512)
                        diag_score(1, 512, 512)
                        diag_mask(0)
                        diag_mask(640)
                        emit_exp(ps[:, :1024], pt[:, 512 * bd : 512 * bd + 1024], 1024)
                    else:
                        diag_score(2, 0, 256)
                        diag_mask(0)
                        diag_score(3, 256, 128)
                        diag_mask(256)
                        emit_exp(
                            ps[:, :384],
                            pt[:, 512 * bd + 1024 : 512 * bd + 1408],
                            384,
                        )

                units = [(p, qi) for qi in reversed(range(_NQB)) for p in range(2)]
                prev = None
                for ui, (p, qi) in enumerate(units):
                    bd = 4 * qi
                    offs = [
                        ("off", b0, min(_CHUNK, bd - b0))
                        for b0 in range(0, bd, _CHUNK)
                    ]
                    diags = [("diag0", bd, 2), ("diag1", bd, 2)]
                    if ui == 0:
                        # consume keys high-to-low: matches the descending
                        # input DMA order, so compute starts immediately
                        chunks = diags + offs[::-1]
                    else:
                        chunks = offs + diags
                    nchunks = len(chunks)
                    pt = ptp.tile([128, 512 * _NKB], bf, tag="pt", name="pt")
                    if qi > 0:
                        emit_z(p, qi - 1)  # consumed two units later
                    last = ui == len(units) - 1
                    if last:
                        # self-paced drain on the spare ztv psum slot: PVs
                        # chase this unit's own exp chunk by chunk
                        me = BUnit(p, qi, pt, tag="ztv")
                        mydone = 0
                    for ci, (kind, b0, nblk) in enumerate(chunks):
                        if prev is not None:
                            prev.emit_upto((ci * prev.nkb) // nchunks)
                        if kind == "off":
                            emit_offdiag_chunk(p, qi, pt, b0, nblk)
                        elif kind == "diag0":
                            emit_diag_chunk(p, qi, pt, 0)
                        else:
                            emit_diag_chunk(p, qi, pt, 1)
                        if ui == 0 and ci == 10:
                            emit_vproj(0)
                        if ui == 1 and ci in (3, 7):
                            emit_vproj(1 if ci == 3 else 2)
                        if last:
                            mydone += nblk
                            me.emit_upto(mydone)
                    if prev is not None:
                        prev.finish()
                    if last:
                        me.finish()
                    else:
                        prev = BUnit(p, qi, pt)

    nc.compile()
    return nc


def _get_nc():
    if "nc" not in _cache:
        _cache["nc"] = _build_nc()
    return _cache["nc"]


def _prepare_in_maps(x, Wq, bq, Wk, bk, Wv, bv):
    bfd = ml_dtypes.bfloat16
    x = np.asarray(x, np.float32)
    Wq = np.asarray(Wq, np.float32)
    bq = np.asarray(bq, np.float32)
    Wk = np.asarray(Wk, np.float32)
    Wv = np.asarray(Wv, np.float32)

    tmv = np.where(
        np.arange(128)[:, None] > np.arange(128)[None, :], _MASK, 0.0
    ).astype(np.float32)
    i128 = np.eye(128, dtype=np.float32)

    in_maps = []
    for c in range(_NC):
        b_idx = c // 2
        heads = (2 * (c % 2), 2 * (c % 2) + 1)
        xT = x[b_idx].T  # [64, 4096]
        x2 = np.concatenate([xT, xT], axis=0)  # [128, 4096]
        zw = np.zeros((_D, 128), np.float32)
        x2v = np.zeros((2, _D + 1, _S), np.float32)
        wv = np.zeros((_D + 1, 34), np.float32)
        for p, h in enumerate(heads):
            hs = slice(h * _Dh, (h + 1) * _Dh)
            Wqh, Wkh, Wvh = Wq[hs], Wk[hs], Wv[hs]
            M = Wkh.T @ Wqh / np.sqrt(_Dh)  # [64, 64]
            zw[:, 64 * p : 64 * p + 64] = M.T
            C = (bq[hs] @ (Wkh @ xT)) / np.sqrt(_Dh)  # [4096]
            expC = np.exp(C).astype(np.float32)
            x2v[p, :_D] = xT * expC[None, :]
            x2v[p, _D] = expC
            wv[:_D, 17 * p : 17 * p + 16] = Wvh.T
            wv[_D, 17 * p + 16] = 1.0
        in_maps.append(
            {
                "x2": x2.astype(bfd),
                "zw": zw.astype(bfd),
                "x2v": x2v.astype(bfd),
                "wv": wv.astype(bfd),
                "tmv": tmv.astype(bfd),
                "i128": i128.astype(bfd),
            }
        )
    return in_maps


def _assemble(results, bv):
    bv = np.asarray(bv, np.float32)
    final = np.empty((_B, _S, _D), np.float32)
    for c in range(_NC):
        b_idx = c // 2
        o = np.asarray(results[c]["out"], np.float32)  # [2, NQB, 128, 512]
        for p in range(2):
            h = 2 * (c % 2) + p
            hs = slice(h * _Dh, (h + 1) * _Dh)
            # qi>=1: four col-group partials; qi=0: two, and the second
            # never covers queries 0..127 (use the first alone there)
            part = o[p, :, 0:17] + o[p, :, 32:49]  # [NQB, 17, 512]
            part[1:] += o[p, 1:, 64:81] + o[p, 1:, 96:113]
            part[0, :, 0:128] = o[p, 0, 0:17, 0:128]
            ot = part.transpose(1, 0, 2).reshape(17, _S)  # [17, S]
            final[b_idx, :, hs] = (ot[:16] / ot[16:17]).T + bv[hs][None, :]
    return final


def _run(in_maps, trace=False, trace_kwargs=None):
    from concourse.bass_utils import run_bass_kernel_spmd

    nc = _get_nc()
    return run_bass_kernel_spmd(
        nc, in_maps, list(range(_NC)), trace=trace, **(trace_kwargs or {})
    )


def kernel(x, Wq, bq, Wk, bk, Wv, bv):
    in_maps = _prepare_in_maps(x, Wq, bq, Wk, bk, Wv, bv)
    res = _run(in_maps)
    return _assemble(res.results, bv)


# revision 64
# speedup vs baseline: 1.2039x; 1.0016x over previous
"""Causal self-attention (B=4, S=4096, D=64, H=4) on 8 TRN2 NeuronCores.

Sharding: 16 (batch, head) pairs, 2 per core (core c -> batch c//2,
heads (2*(c%2), 2*(c%2)+1)). Each core runs fused attention for its 2
pairs; no cross-core communication.

Per-core program (SPMD). Key structure vs a naive port:
  - Q/K projections are folded into the score matmul: host precomputes
    M_h = Wk_h^T Wq_h / sqrt(Dh) per head, the kernel computes
    Z = M x once (K=64) and scores S^T = x^T Z directly (K=64
    contraction, 2-band row tiling at partition offsets 0/64).
    Query-side bias terms are softmax-invariant and dropped; the
    key-side bias term exp(C_k), C_k = bq.(Wk x_k)/sqrt(Dh), is folded
    into a host-prescaled copy of x used for the V projection (x2v),
    and bv is added on the host after the final division.
  - scores are computed TRANSPOSED (key position on partitions) so P@V
    needs no transpose; the softmax denominator comes from an extra
    "ones" column in the V projection (which also carries exp(C_k)).
  - exp of the score matrix is split between the ACT engine (exact Exp)
    and the DVE (Schraudolph-style bit-trick: y_i16 = trunc(A*s + B)
    bitcast to bf16 ~ exp(s), max rel err ~3.3%), load-balanced, since
    ACT alone is a ~113us floor at 1.2 GHz x 128 lanes.
  - causal mask: -30 additive on the 4 diagonal 128x128 tiles per
    query superblock, applied by the PE itself as an accumulating
    matmul (stationary = I, moving = -30*strict_lower_tri), freeing
    ACT/DVE. Below-diagonal garbage is excluded from PV by qoff.
  - PV matmuls are 2-way column-tiled (out M=17 <= 32, col groups at
    partition 0/32 of one PSUM bank); the two partial sums are reduced
    and copied to SBUF in one DVE scalar_tensor_tensor, then DMA'd out.
  - The PE stream is kept dense (software-pipelined PV of the previous
    unit between score chunks) so the HAM clock gate stays at 2.4 GHz.
  - output: [2 pairs, 17, 4096] f32 = unnormalized O^T rows 0..15 plus
    the softmax denominator in row 16; division + bv happen on host.
"""

import numpy as np
import ml_dtypes

_B, _S, _D = 4, 4096, 64
_H, _Dh = 4, 16
_NC = 8
_NQB = _S // 512  # 8 query superblocks of 512
_NKB = _S // 128  # 32 key blocks of 128
_CHUNK = 2  # off-diag key-blocks per psum chunk (2 banks, 3-deep pipeline)
_MASK = -30.0
_EXP_A = 184.6649652337873  # 128/ln2
_EXP_B = 16250.9  # 128*127 - c, calibrated for trunc, min-max rel err

# diag chunk psum packing: block j at col offs[j], stream length lens[j],
# starting at query qstart[j] (within the 512-superblock); the causal
# triangle of block j sits at col moff[j].
_D_OFF = (0, 512, 1024, 1280)
_D_LEN = (512, 512, 256, 128)
_D_QS = (0, 0, 256, 384)
_D_MOFF = (0, 640, 1024, 1280)
# PV moving slice for diag block j: pt col offset (rel. to diag region
# start), length, and po4 query offset
_D_PV = ((0, 512, 0), (640, 384, 128), (1024, 256, 256), (1280, 128, 384))

_cache = {}


def _build_nc():
    import concourse.tile as tile
    from concourse import bacc, mybir

    bf = mybir.dt.bfloat16
    i16 = mybir.dt.int16
    f32 = mybir.dt.float32
    Exp = mybir.ActivationFunctionType.Exp
    MUL = mybir.AluOpType.mult
    ADD = mybir.AluOpType.add

    nc = bacc.Bacc("TRN2", target_bir_lowering=False, debug=False, num_devices=_NC)
    x2_d = nc.dram_tensor("x2", [128, _S], bf, kind="ExternalInput").ap()
    zw_d = nc.dram_tensor("zw", [_D, 128], bf, kind="ExternalInput").ap()
    x2v_d = nc.dram_tensor("x2v", [2, _D + 1, _S], bf, kind="ExternalInput").ap()
    wv_d = nc.dram_tensor("wv", [_D + 1, 34], bf, kind="ExternalInput").ap()
    tmv_d = nc.dram_tensor("tmv", [128, 128], bf, kind="ExternalInput").ap()
    i128_d = nc.dram_tensor("i128", [128, 128], bf, kind="ExternalInput").ap()
    # per (pair, superblock): rows 0-16 = PV partial sum of even key
    # blocks (16 v-dims + denominator), rows 32-48 = odd partial; host
    # adds the two partials and divides.
    out_d = nc.dram_tensor("out", [2, _NQB, 128, 512], f32, kind="ExternalOutput").ap()

    # static greedy load balance between ACT ("scalar") and DVE ("vector")
    load = {"a": 0.0, "v": 0.0}

    def exp_cost(eng, cols):
        return (cols + 430) * 0.833 if eng == "a" else (cols + 300) * 1.042

    def pick_engine(cols):
        ca, cv = exp_cost("a", cols), exp_cost("v", cols)
        e = "a" if load["a"] + ca <= load["v"] + cv else "v"
        load[e] += exp_cost(e, cols)
        return e

    with tile.TileContext(nc) as tc:
        with tc.tile_pool(name="singles", bufs=1) as singles:
            x2 = singles.tile([128, _S], bf, tag="x2")
            zw = singles.tile([_D, 128], bf, tag="zw")
            x2v = [
                singles.tile([_D + 1, _S], bf, tag=f"x2v{p}", name=f"x2v{p}")
                for p in range(2)
            ]
            wv = singles.tile([_D + 1, 34], bf, tag="wv")
            tmv = singles.tile([128, 128], bf, tag="tmv")
            i128 = singles.tile([128, 128], bf, tag="i128")
            z2 = [
                singles.tile([128, _S], bf, tag=f"z2{p}", name=f"z2{p}")
                for p in range(2)
            ]
            # V layout: slot (b, p) at index 2b+p, 32 cols each: 17 real
            # (16 v-dims + denominator) + 15 zeros. The even PV col-group
            # uses the full 32-wide slot so its output also zero-fills po
            # rows 17..31 (which the drain copy reads).
            vall = singles.tile([128, 2 * _NKB, 32], bf, tag="vall")
            nc.vector.memset(vall[:], 0.0)

            # spread input DMAs over 4 DGE queues; per queue: small consts
            # first, then an x2 quarter (needed by the first scores), then
            # an x2v half (needed by the deferred V projection)
            # 3 usable DGE queues. x2 goes in 8 pieces, HIGH columns first
            # (the first unit consumes keys high-to-low and the Z projection
            # needs the last 512 columns), so the PE can start ~3us in.
            # x2v (only needed by the deferred V projection) follows.
            qs = [nc.sync, nc.gpsimd, nc.scalar]
            nc.sync.dma_start(out=zw[:], in_=zw_d)
            nc.gpsimd.dma_start(out=wv[:], in_=wv_d)
            nc.scalar.dma_start(out=tmv[:], in_=tmv_d)
            nc.scalar.dma_start(out=i128[:], in_=i128_d)
            for i, s in enumerate(reversed(range(8))):
                qs[i % 3].dma_start(
                    out=x2[:, 512 * s : 512 * (s + 1)],
                    in_=x2_d[:, 512 * s : 512 * (s + 1)],
                )
            for p in range(2):
                for c in range(2):
                    qs[(2 * p + c) % 3].dma_start(
                        out=x2v[p][:, 2048 * c : 2048 * (c + 1)],
                        in_=x2v_d[p][:, 2048 * c : 2048 * (c + 1)],
                    )

            def copy_psum(dst, src, cols):
                """psum->sbuf copy on the less-loaded of ACT/DVE."""
                e = pick_engine(cols)
                if e == "a":
                    nc.scalar.copy(dst, src)
                else:
                    nc.vector.tensor_copy(dst, src)

            # one PSUM pool: tag "sc" 3x2 banks + "po" 1 + "ztv" 1 = 8 banks
            with (
                tc.tile_pool(name="psp", bufs=3, space="PSUM") as psp,
                tc.tile_pool(name="ptp", bufs=2) as ptp,
                tc.tile_pool(name="stg", bufs=3) as stg,
            ):
                def emit_z(p, qi):
                    """Z2_p chunk qi: Z = M x, written twice (col-tiled) so
                    the copy lands band-replicated at partitions 0/64."""
                    csl = slice(512 * qi, 512 * (qi + 1))
                    zt = psp.tile([128, 512], f32, tag="ztv", name="zt", bufs=1)
                    nc.tensor.matmul(
                        zt[0:64, :],
                        zw[:, 64 * p : 64 * p + 64],
                        x2[0:64, csl],
                        start=True,
                        stop=True,
                        tile_position=(0, 0),
                    )
                    nc.tensor.matmul(
                        zt[64:128, :],
                        zw[:, 64 * p : 64 * p + 64],
                        x2[0:64, csl],
                        start=True,
                        stop=True,
                        tile_position=(0, 64),
                    )
                    copy_psum(z2[p][:, csl], zt[:], 512)

                # Pre-warm the PE: the HAM clock gate needs ~3.4us of
                # sustained activity to release the 1.2 GHz throttle. These
                # dummy matmuls run while the input DMAs land, so real work
                # starts at 2.4 GHz.
                warm = singles.tile([128, 16], bf, tag="warm")
                warm2 = singles.tile([128, 512], bf, tag="warm2")
                nc.vector.memset(warm[:], 0.0)
                nc.vector.memset(warm2[:], 0.0)
                wps = psp.tile([128, 512], f32, tag="po", name="wps", bufs=1)
                for _ in range(14):
                    nc.tensor.matmul(
                        wps[0:16, :], warm[:], warm2[:], start=True, stop=True
                    )

                # Z chunks for the first units; the remaining Z chunks are
                # emitted one per unit inside the loop.
                emit_z(0, _NQB - 1)
                emit_z(1, _NQB - 1)

                # V projection: out[k, col] = sum_d x2v_p[d,k] * wv[d,col];
                # col 16 of each pair block is the (exp(C_k)) denominator.
                # Deferred into the first unit's emission so the x2v input
                # DMA overlaps with the first score chunks.
                _vt_nb = [15, 15, 2]

                def emit_vproj(t):
                    nb = _vt_nb[t]
                    b0v = sum(_vt_nb[:t])
                    vt = psp.tile([128, 512], f32, tag="ztv", name="vt", bufs=1)
                    for i in range(nb):
                        b = b0v + i
                        for p in range(2):
                            nc.tensor.matmul(
                                vt[:, 34 * i + 17 * p : 34 * i + 17 * p + 17],
                                x2v[p][:, 128 * b : 128 * (b + 1)],
                                wv[:, 17 * p : 17 * p + 17],
                                start=True,
                                stop=True,
                                skip_group_check=True,
                            )
                    copy_psum(
                        vall[:, 2 * b0v : 2 * (b0v + nb), 0:17],
                        vt[:, : 34 * nb].rearrange("p (b c) -> p b c", c=17),
                        34 * nb,
                    )

                def emit_exp(ps_ap, pt_ap, cols):
                    e = pick_engine(cols)
                    if e == "a":
                        nc.scalar.activation(out=pt_ap, in_=ps_ap, func=Exp)
                    else:
                        nc.vector.tensor_scalar(
                            pt_ap.bitcast(i16),
                            ps_ap,
                            _EXP_A,
                            _EXP_B,
                            MUL,
                            ADD,
                        )

                out_q = [nc.sync, nc.gpsimd]

                class BUnit:
                    """PV matmuls (4-way col-tiled partial sums; the host
                    adds the partials) + drain copy + output DMA, emitted
                    incrementally so they interleave with the next unit's
                    score chunks."""

                    n_out = [0]

                    def __init__(self, p, qi, pt, tag="po"):
                        self.p, self.qi, self.pt = p, qi, pt
                        self.nkb = 4 * qi + 4
                        self.done = 0
                        self.po = psp.tile([128, 512], f32, tag=tag, name="po", bufs=1)

                    def emit_upto(self, k):
                        # col-group count: 4 for qi>=1, 2 for the all-diag
                        # qi=0 units (their odd groups would leave column
                        # gaps with 4-way tiling)
                        ng = 4 if self.qi >= 1 else 2
                        for b in range(self.done, min(k, self.nkb)):
                            j = b - 4 * self.qi
                            if j < 0:
                                off, ln, qo = 512 * b - 2048 * self.qi, 512, 0
                            else:
                                off, ln, qo = _D_PV[j]
                                if self.qi == 0 and j == 1:
                                    # cover queries 0..127 too (garbage there;
                                    # host ignores) so the drain copy's read
                                    # region is fully written this generation
                                    off, ln, qo = 512, 512, 0
                            base = 2048 * self.qi + off
                            msl = self.pt[:, base : base + ln]
                            vsl = vall[:, 2 * b + self.p]
                            # each col-group uses the 32-wide zero-padded
                            # stationary so its full 32-row band is written;
                            # group 0 carries the sim's group bookkeeping
                            g = b % ng
                            nc.tensor.matmul(
                                self.po[32 * g : 32 * g + 32, qo:512],
                                vsl,
                                msl,
                                start=(b < ng),
                                stop=(b == self.nkb - ng),
                                skip_group_check=(g != 0),
                                tile_position=(0, 32 * g),
                            )
                        self.done = max(self.done, min(k, self.nkb))

                    def finish(self):
                        self.emit_upto(self.nkb)
                        nrow = 128 if self.qi >= 1 else 64
                        ost = stg.tile([128, 512], f32, tag="ost", name="ost")
                        copy_psum(ost[0:nrow, :], self.po[0:nrow, :], 512)
                        h = nrow // 2
                        nc.sync.dma_start(
                            out=out_d[self.p][self.qi][0:h], in_=ost[0:h, :]
                        )
                        nc.gpsimd.dma_start(
                            out=out_d[self.p][self.qi][h:nrow], in_=ost[h:nrow, :]
                        )

                def emit_offdiag_chunk(p, qi, pt, b0, nblk):
                    ps = psp.tile([128, 512 * _CHUNK], f32, tag="sc", name="ps")
                    for t in range(nblk):
                        b = b0 + t
                        g = 64 * (b % 2)
                        nc.tensor.matmul(
                            ps[:, 512 * t : 512 * (t + 1)],
                            x2[g : g + 64, 128 * b : 128 * (b + 1)],
                            z2[p][g : g + 64, 512 * qi : 512 * (qi + 1)],
                            start=True,
                            stop=True,
                            tile_position=(g, 0),
                        )
                    emit_exp(
                        ps[:, : 512 * nblk],
                        pt[:, 512 * b0 : 512 * (b0 + nblk)],
                        512 * nblk,
                    )

                def emit_diag_chunk(p, qi, pt, half):
                    """Diagonal scores in two 2-bank chunks: half 0 = blocks
                    j0 (full 512) + j1 (full 512, garbage below the
                    staircase, PV skips it); half 1 = j2 (256) + j3 (128).
                    Masks: triangle of block j accumulates right after its
                    score; within a shared bank the mask must precede the
                    next score's start=True re-marking."""
                    bd = 4 * qi
                    ps = psp.tile([128, 512 * _CHUNK], f32, tag="sc", name="ps")

                    def diag_score(j, off, ln):
                        b = bd + j
                        g = 64 * (b % 2)
                        qs = 512 * qi + _D_QS[j]
                        nc.tensor.matmul(
                            ps[:, off : off + ln],
                            x2[g : g + 64, 128 * b : 128 * (b + 1)],
                            z2[p][g : g + 64, qs : qs + ln],
                            start=True,
                            stop=True,
                            tile_position=(g, 0),
                        )

                    def diag_mask(off):
                        nc.tensor.matmul(
                            ps[:, off : off + 128],
                            i128[:],
                            tmv[:, 0:128],
                            start=False,
                            stop=False,
                            skip_group_check=True,
                        )

                    if half == 0:
                        diag_score(0, 0, 512)
                        diag_score(1, 512, 512)
                        diag_mask(0)
                        diag_mask(640)
                        emit_exp(ps[:, :1024], pt[:, 512 * bd : 512 * bd + 1024], 1024)
                    else:
                        diag_score(2, 0, 256)
                        diag_mask(0)
                        diag_score(3, 256, 128)
                        diag_mask(256)
                        emit_exp(
                            ps[:, :384],
                            pt[:, 512 * bd + 1024 : 512 * bd + 1408],
                            384,
                        )

                units = [(p, qi) for qi in reversed(range(_NQB)) for p in range(2)]
                prev = None
                for ui, (p, qi) in enumerate(units):
                    bd = 4 * qi
                    offs = [
                        ("off", b0, min(_CHUNK, bd - b0))
                        for b0 in range(0, bd, _CHUNK)
                    ]
                    diags = [("diag0", bd, 2), ("diag1", bd, 2)]
                    if ui == 0:
                        # consume keys high-to-low: matches the descending
                        # input DMA order, so compute starts immediately
                        chunks = diags + offs[::-1]
                    else:
                        chunks = offs + diags
                    nchunks = len(chunks)
                    pt = ptp.tile([128, 512 * _NKB], bf, tag="pt", name="pt")
                    if qi > 0:
                        emit_z(p, qi - 1)  # consumed two units later
                    last = ui == len(units) - 1
                    if last:
                        # self-paced drain on the spare ztv psum slot: PVs
                        # chase this unit's own exp chunk by chunk
                        me = BUnit(p, qi, pt, tag="ztv")
                        mydone = 0
                    for ci, (kind, b0, nblk) in enumerate(chunks):
                        if prev is not None:
                            prev.emit_upto((ci * prev.nkb) // nchunks)
                        if kind == "off":
                            emit_offdiag_chunk(p, qi, pt, b0, nblk)
                        elif kind == "diag0":
                            emit_diag_chunk(p, qi, pt, 0)
                        else:
                            emit_diag_chunk(p, qi, pt, 1)
                        if ui == 0 and ci == 10:
                            emit_vproj(0)
                        if ui == 1 and ci in (3, 7):
                            emit_vproj(1 if ci == 3 else 2)
                        if last:
                            mydone += nblk
                            me.emit_upto(mydone)
                    if prev is not None:
                        prev.finish()
                    if last:
                        me.finish()
                    else:
                        prev = BUnit(p, qi, pt)

    nc.compile()
    return nc


def _get_nc():
    if "nc" not in _cache:
        _cache["nc"] = _build_nc()
    return _cache["nc"]


def _prepare_in_maps(x, Wq, bq, Wk, bk, Wv, bv):
    bfd = ml_dtypes.bfloat16
    x = np.asarray(x, np.float32)
    Wq = np.asarray(Wq, np.float32)
    bq = np.asarray(bq, np.float32)
    Wk = np.asarray(Wk, np.float32)
    Wv = np.asarray(Wv, np.float32)

    tmv = np.where(
        np.arange(128)[:, None] > np.arange(128)[None, :], _MASK, 0.0
    ).astype(np.float32)
    i128 = np.eye(128, dtype=np.float32)

    in_maps = []
    for c in range(_NC):
        b_idx = c // 2
        heads = (2 * (c % 2), 2 * (c % 2) + 1)
        xT = x[b_idx].T  # [64, 4096]
        x2 = np.concatenate([xT, xT], axis=0)  # [128, 4096]
        zw = np.zeros((_D, 128), np.float32)
        x2v = np.zeros((2, _D + 1, _S), np.float32)
        wv = np.zeros((_D + 1, 34), np.float32)
        for p, h in enumerate(heads):
            hs = slice(h * _Dh, (h + 1) * _Dh)
            Wqh, Wkh, Wvh = Wq[hs], Wk[hs], Wv[hs]
            M = Wkh.T @ Wqh / np.sqrt(_Dh)  # [64, 64]
            zw[:, 64 * p : 64 * p + 64] = M.T
            C = (bq[hs] @ (Wkh @ xT)) / np.sqrt(_Dh)  # [4096]
            expC = np.exp(C).astype(np.float32)
            x2v[p, :_D] = xT * expC[None, :]
            x2v[p, _D] = expC
            wv[:_D, 17 * p : 17 * p + 16] = Wvh.T
            wv[_D, 17 * p + 16] = 1.0
        in_maps.append(
            {
                "x2": x2.astype(bfd),
                "zw": zw.astype(bfd),
                "x2v": x2v.astype(bfd),
                "wv": wv.astype(bfd),
                "tmv": tmv.astype(bfd),
                "i128": i128.astype(bfd),
            }
        )
    return in_maps


def _assemble(results, bv):
    bv = np.asarray(bv, np.float32)
    final = np.empty((_B, _S, _D), np.float32)
    for c in range(_NC):
        b_idx = c // 2
        o = np.asarray(results[c]["out"], np.float32)  # [2, NQB, 128, 512]
        for p in range(2):
            h = 2 * (c % 2) + p
            hs = slice(h * _Dh, (h + 1) * _Dh)
            # qi>=1: four col-group partials; qi=0: two, and the second
            # never covers queries 0..127 (use the first alone there)
            part = o[p, :, 0:17] + o[p, :, 32:49]  # [NQB, 17, 512]
            part[1:] += o[p, 1:, 64:81] + o[p, 1:, 96:113]
            part[0, :, 0:128] = o[p, 0, 0:17, 0:128]
            ot = part.transpose(1, 0, 2).reshape(17, _S)  # [17, S]
            final[b_idx, :, hs] = (ot[:16] / ot[16:17]).T + bv[hs][None, :]
    return final


def _run(in_maps, trace=False, trace_kwargs=None):
    from concourse.bass_utils import run_bass_kernel_spmd

    nc = _get_nc()
    return run_bass_kernel_spmd(
        nc, in_maps, list(range(_NC)), trace=trace, **(trace_kwargs or {})
    )


def kernel(x, Wq, bq, Wk, bk, Wv, bv):
    in_maps = _prepare_in_maps(x, Wq, bq, Wk, bk, Wv, bv)
    res = _run(in_maps)
    return _assemble(res.results, bv)
